# revision 14
# baseline (speedup 1.0000x reference)
"""Multi-head causal attention (SEQ=4096, D=1024, H=16, DK=64) on 8 TRN2
NeuronCores, tensor-parallel over heads (2 heads/core). Self-contained.

Per-core pipeline:
  1. Projections: Qh^T/Kh^T/Vh^T = W.T @ X^T (X^T passed pre-transposed from
     host, 1/sqrt(dk) folded into W_Q host-side). bf16 copies kept for stats.
  2. Stats pass: S = Qh^T.T @ Kh^T in bf16, causal mask added via
     identity-matmul of a -1e9 mask tile, row-max reduced (bf16 error is
     harmless: softmax is shift-invariant, m only needs |m - max| << 80).
  3. S^T pass in exact fp32: S^T[kc,q] = [Kh;1].T @ [Qh;-m] (the max
     subtraction rides the contraction as a 65th row), mask via identity
     matmul, then ACT exp -> P^T. AV in f32r: O^T = [Vh|1].T-style
     ones-augmented Vh gives l = sum(exp) as row 64 of the PSUM accumulator.
  4. R = rank-2 broadcast of 1/l over head halves (PE), C^T scaled (DVE),
     Y_partial = C^T.T @ W_O_rows in f32r, DMA out. Host sums 8 partials.
"""

import os
import sys

sys.path.insert(0, "/opt/trn_rl_repo")

import numpy as np
import ml_dtypes

import concourse.bass as bass
import concourse.mybir as mybir
import concourse.tile as tile
from concourse.bass_utils import run_bass_kernel_spmd
from concourse.masks import make_identity

P = 128
S = 4096
D = 1024
DK = 64
NH = 2  # heads per core
NCORES = 8
NEG = -1.0e9
F32 = mybir.dt.float32
F32R = mybir.dt.float32r
BF16 = mybir.dt.bfloat16
EXP = mybir.ActivationFunctionType.Exp

_ctr = [0]


def _split_waits(nc, max_waits=1):
    """walrus rejects >1 sem-wait per instruction; move extras onto
    preceding same-engine NOPs (engine streams are program-ordered)."""
    for f in nc.m.functions:
        for bb in f.blocks:
            insts = bb.instructions
            new = []
            changed = False
            for inst in insts:
                si = inst.sync_info
                if si is not None and si.on_wait and len(si.on_wait) > max_waits:
                    waits = list(si.on_wait)
                    extra, keep = waits[:-max_waits], waits[-max_waits:]
                    for i in range(0, len(extra), max_waits):
                        _ctr[0] += 1
                        new.append(
                            mybir.InstNoOp(
                                name=f"waitsplit-{_ctr[0]}",
                                engine=inst.engine,
                                ins=[],
                                outs=[],
                                sync_info=mybir.SyncInfo(
                                    on_wait=extra[i : i + max_waits], on_update=[]
                                ),
                            )
                        )
                    inst.sync_info = mybir.SyncInfo(
                        on_wait=keep, on_update=list(si.on_update)
                    )
                    changed = True
                new.append(inst)
            if changed:
                bb.instructions = new


def build(nc: bass.Bass, causal: bool = True):
    stages = int(os.environ.get("ATTN_STAGES", "6"))
    NB = S // 512  # 8   512-wide blocks
    QB = S // P  # 32  128-wide q blocks
    DC = D // P  # 8   128-deep contraction chunks

    qT = nc.dram_tensor("qT", [D, S], F32, kind="ExternalInput")
    kT = nc.dram_tensor("kT", [D, S], F32, kind="ExternalInput")
    vT = nc.dram_tensor("vT", [D, S], F32, kind="ExternalInput")
    wq = nc.dram_tensor("wq", [D, NH * DK], F32, kind="ExternalInput")
    wk = nc.dram_tensor("wk", [D, NH * DK], F32, kind="ExternalInput")
    wv = nc.dram_tensor("wv", [D, NH * DK], F32, kind="ExternalInput")
    wo = nc.dram_tensor("wo", [NH * DK, D], F32R, kind="ExternalInput")
    maskf = nc.dram_tensor("maskf", [4, P, 512], F32R, kind="ExternalInput")
    maskb = nc.dram_tensor("maskb", [4, P, 512], BF16, kind="ExternalInput")
    y0 = nc.dram_tensor("y0", [S, D], F32, kind="ExternalOutput")
    y1 = nc.dram_tensor("y1", [S, D], F32, kind="ExternalOutput")

    with tile.TileContext(nc) as tc:
        import contextlib

        ctx = contextlib.ExitStack()
        with ctx:
            const = ctx.enter_context(tc.tile_pool(name="const", bufs=1))
            big = ctx.enter_context(tc.tile_pool(name="big", bufs=1))
            stream = ctx.enter_context(tc.tile_pool(name="stream", bufs=3))
            ptp = ctx.enter_context(tc.tile_pool(name="ptp", bufs=3))
            ypool = ctx.enter_context(tc.tile_pool(name="ypool", bufs=2))
            smalls = ctx.enter_context(tc.tile_pool(name="smalls", bufs=2))
            ps_proj = ctx.enter_context(
                tc.tile_pool(name="ps_proj", bufs=2, space="PSUM")
            )
            ps_stat = ctx.enter_context(
                tc.tile_pool(name="ps_stat", bufs=2, space="PSUM")
            )
            ps_st = ctx.enter_context(tc.tile_pool(name="ps_st", bufs=2, space="PSUM"))
            ps_vtr = ctx.enter_context(
                tc.tile_pool(name="ps_vtr", bufs=1, space="PSUM")
            )
            ps_ot = ctx.enter_context(tc.tile_pool(name="ps_ot", bufs=1, space="PSUM"))

            # ---- constants ----
            ident = const.tile([P, P], F32)
            make_identity(nc, ident[:])
            ident_b = const.tile([P, P], BF16)
            nc.vector.tensor_copy(ident_b[:], ident[:])
            ident_r = const.tile([P, P], F32R)
            nc.vector.tensor_copy(ident_r[:], ident[:])

            wq_sb = const.tile([P, DC, P], F32, tag="wq")
            wk_sb = const.tile([P, DC, P], F32, tag="wk")
            wv_sb = const.tile([P, DC, P], F32, tag="wv")
            nc.sync.dma_start(wq_sb[:], wq.rearrange("(o p) m -> p o m", p=P))
            nc.sync.dma_start(wk_sb[:], wk.rearrange("(o p) m -> p o m", p=P))
            nc.sync.dma_start(wv_sb[:], wv.rearrange("(o p) m -> p o m", p=P))
            wo_sb = const.tile([P, D], F32R, tag="wo")
            nc.sync.dma_start(wo_sb[:], wo[:])

            mf_sb = const.tile([P, 4, 512], F32R, tag="mf")
            mb_sb = const.tile([P, 4, 512], BF16, tag="mb")
            nc.sync.dma_start(mf_sb[:], maskf.rearrange("o p f -> p o f"))
            nc.sync.dma_start(mb_sb[:], maskb.rearrange("o p f -> p o f"))

            # ---- persistent activations ----
            qhT = [big.tile([P, S], F32, tag=f"qhT{h}", name=f"qhT{h}") for h in range(NH)]
            khT = [big.tile([P, S], F32, tag=f"khT{h}", name=f"khT{h}") for h in range(NH)]
            qhT_bf = big.tile([P, S], BF16, tag="qhT_bf")  # both heads packed
            khT_bf = big.tile([P, S], BF16, tag="khT_bf")
            vh = [big.tile([P, QB, DK + 1], F32R, tag=f"vh{h}", name=f"vh{h}") for h in range(NH)]
            ct = big.tile([P, S], F32R, tag="ct")
            mcol = [big.tile([P, QB], F32, tag=f"mcol{h}", name=f"mcol{h}") for h in range(NH)]
            # lmat rows p0/p32: per-head l (sum of exp) as rows
            lmat = big.tile([P, S], F32, tag="lmat")
            lcol = [big.tile([P, QB], F32, tag=f"lcol{h}", name=f"lcol{h}") for h in range(NH)]
            rcol = [big.tile([P, QB], F32, tag=f"rcol{h}", name=f"rcol{h}") for h in range(NH)]

            ones_qb = const.tile([P, QB], F32, tag="ones_qb")
            nc.any.memset(ones_qb[:], 1.0)
            for h in range(NH):
                nc.any.memset(khT[h][DK : DK + 1, :], 1.0)  # ones row (row 64)
                nc.vector.tensor_copy(vh[h][:, :, DK], ones_qb[:])  # ones col


            # ---- stage 1: projections ----
            for t_idx, (xdram, w_sb) in enumerate(
                [(qT, wq_sb), (kT, wk_sb), (vT, wv_sb)]
            ):
                for nb in range(NB):
                    ps = ps_proj.tile([P, 512], F32, tag="proj")
                    for dc in range(DC):
                        xt = stream.tile([P, 512], F32, tag="xin")
                        nc.sync.dma_start(
                            xt[:],
                            xdram[dc * P : (dc + 1) * P, nb * 512 : (nb + 1) * 512],
                        )
                        nc.tensor.matmul(
                            ps[:],
                            w_sb[:, dc, :],
                            xt[:],
                            start=(dc == 0),
                            stop=(dc == DC - 1),
                        )
                    if t_idx == 0:
                        for h in range(NH):
                            nc.vector.tensor_copy(
                                qhT[h][0:DK, nb * 512 : (nb + 1) * 512],
                                ps[h * DK : (h + 1) * DK, :],
                            )
                        nc.scalar.copy(qhT_bf[:, nb * 512 : (nb + 1) * 512], ps[:])
                    elif t_idx == 1:
                        for h in range(NH):
                            nc.vector.tensor_copy(
                                khT[h][0:DK, nb * 512 : (nb + 1) * 512],
                                ps[h * DK : (h + 1) * DK, :],
                            )
                        nc.scalar.copy(khT_bf[:, nb * 512 : (nb + 1) * 512], ps[:])
                    else:
                        # Vh^T chunk -> per-head transposes into vh [kc, dk]
                        vtmp = stream.tile([P, 512], F32, tag="vtmp")
                        nc.vector.tensor_copy(vtmp[:], ps[:])
                        for j in range(4):
                            kcb = nb * 4 + j
                            for h in range(NH):
                                pst = ps_vtr.tile([P, P], F32, tag="vtr")
                                nc.tensor.transpose(
                                    pst[0:P, 0:DK],
                                    vtmp[h * DK : (h + 1) * DK, j * P : (j + 1) * P],
                                    ident[h * DK : (h + 1) * DK, h * DK : (h + 1) * DK],
                                )
                                nc.vector.tensor_copy(
                                    vh[h][:, kcb, 0:DK], pst[0:P, 0:DK]
                                )

            # ---- stage 2: stats (bf16), per head ----
            for h in range(NH if stages >= 2 else 0):
                hs = slice(h * DK, (h + 1) * DK)
                for qb in range(QB):
                    kmax = qb // 4 + 1 if causal else NB
                    mpart = smalls.tile([P, NB], F32, tag="mpart")
                    for kc in range(kmax):
                        ps = ps_stat.tile([P, 512], F32, tag="stat")
                        diag = causal and (kc == qb // 4)
                        nc.tensor.matmul(
                            ps[:],
                            qhT_bf[hs, qb * P : (qb + 1) * P],
                            khT_bf[hs, kc * 512 : (kc + 1) * 512],
                            start=True,
                            stop=not diag,
                        )
                        if diag:
                            nc.tensor.matmul(
                                ps[:],
                                ident_b[:],
                                mb_sb[:, qb % 4, :],
                                start=False,
                                stop=True,
                            )
                        nc.vector.reduce_max(
                            mpart[:, kc : kc + 1], ps[:], axis=mybir.AxisListType.X
                        )
                    # final combine, negated: mcol = -rowmax
                    nc.vector.tensor_reduce(
                        mcol[h][:, qb : qb + 1],
                        mpart[:, 0:kmax],
                        axis=mybir.AxisListType.X,
                        op=mybir.AluOpType.max,
                        negate=True,
                    )
                    # scatter -m into the augmented row of qhT (free-dim row)
                    nc.sync.dma_start(
                        qhT[h][DK : DK + 1, qb * P : (qb + 1) * P],
                        mcol[h][:, qb : qb + 1],
                    )

            # ---- stage 3: S^T (fp32 exact) + exp + AV (f32r) ----
            for h in range(NH if stages >= 3 else 0):
                for nb in range(NB):
                    nkc = 4 * (nb + 1) if causal else QB
                    po = ps_ot.tile([P, 512], F32, tag="ot")
                    for kc in range(nkc):
                        ps = ps_st.tile([P, 512], F32, tag="st")
                        diag = causal and (kc >= 4 * nb)
                        nc.tensor.matmul(
                            ps[:],
                            khT[h][0 : DK + 1, kc * P : (kc + 1) * P],
                            qhT[h][0 : DK + 1, nb * 512 : (nb + 1) * 512],
                            start=True,
                            stop=not diag,
                        )
                        if diag:
                            nc.tensor.matmul(
                                ps[:],
                                ident_r[:],
                                mf_sb[:, kc - 4 * nb, :],
                                start=False,
                                stop=True,
                            )
                        pt = ptp.tile([P, 512], F32R, tag="pt")
                        nc.scalar.activation(pt[:], ps[:], EXP)
                        nc.tensor.matmul(
                            po[0 : DK + 1, :],
                            vh[h][:, kc, :],
                            pt[:],
                            start=(kc == 0),
                            stop=(kc == nkc - 1),
                        )
                    nc.vector.tensor_copy(
                        ct[h * DK : (h + 1) * DK, nb * 512 : (nb + 1) * 512],
                        po[0:DK, :],
                    )
                    nc.vector.tensor_copy(
                        lmat[32 * h : 32 * h + 1, nb * 512 : (nb + 1) * 512],
                        po[DK : DK + 1, :],
                    )

            # ---- stage 4: 1/l columns + W_O with per-partition 1/l scale ----
            if stages >= 4:
                for h in range(NH):
                    for qb in range(QB):
                        nc.sync.dma_start(
                            lcol[h][:, qb : qb + 1],
                            lmat[32 * h : 32 * h + 1, qb * P : (qb + 1) * P],
                        )
                    nc.vector.reciprocal(rcol[h][:], lcol[h][:])
            ID = mybir.ActivationFunctionType.Identity
            for qc in range(QB if stages >= 5 else 0):
                for h, ydram in ((0, y0), (1, y1)):
                    ysb = ypool.tile([P, D], F32, tag="ysb")
                    for eb in range(2):
                        psy = ps_proj.tile([P, 512], F32, tag="proj")
                        nc.tensor.matmul(
                            psy[:],
                            ct[h * DK : (h + 1) * DK, qc * P : (qc + 1) * P],
                            wo_sb[h * DK : (h + 1) * DK, eb * 512 : (eb + 1) * 512],
                            start=True,
                            stop=True,
                        )
                        nc.scalar.activation(
                            ysb[:, eb * 512 : (eb + 1) * 512],
                            psy[:],
                            ID,
                            scale=rcol[h][:, qc : qc + 1],
                        )
                    nc.sync.dma_start(ydram[qc * P : (qc + 1) * P, :], ysb[:])

    _split_waits(nc)
    return nc


_cache = {}


def _get_nc(causal: bool):
    if causal not in _cache:
        nc = bass.Bass(trn_type="TRN2")
        build(nc, causal=causal)
        _cache[causal] = nc
    return _cache[causal]


def _host_masks():
    p = np.arange(P)[:, None]
    j = np.arange(512)[None, :]
    maskf = np.zeros((4, P, 512), dtype=np.float32)
    maskb = np.zeros((4, P, 512), dtype=np.float32)
    for o in range(4):
        # S^T tile [kc, q]: mask where kc_global > q_global  (p + 128*o > j)
        maskf[o] = np.where(p + P * o > j, NEG, 0.0).astype(np.float32)
        # stats tile [q, kc]: mask where kc_global > q_global (j > p + 128*o)
        maskb[o] = np.where(j > p + P * o, NEG, 0.0).astype(np.float32)
    return maskf, maskb.astype(ml_dtypes.bfloat16)


LAST_EXEC_NS = None


def kernel(Q, K, V, W_Q, W_K, W_V, W_O, mask):
    global LAST_EXEC_NS
    causal = bool(np.asarray(mask).item())
    nc = _get_nc(causal)

    Q = np.asarray(Q, dtype=np.float32)
    K = np.asarray(K, dtype=np.float32)
    V = np.asarray(V, dtype=np.float32)
    W_Q = np.asarray(W_Q, dtype=np.float32)
    W_K = np.asarray(W_K, dtype=np.float32)
    W_V = np.asarray(W_V, dtype=np.float32)
    W_O = np.asarray(W_O, dtype=np.float32)

    qTh = np.ascontiguousarray(Q.T)
    kTh = np.ascontiguousarray(K.T)
    vTh = np.ascontiguousarray(V.T)
    maskf, maskb = _host_masks()

    scale = 1.0 / np.sqrt(DK).astype(np.float32)
    in_maps = []
    for c in range(NCORES):
        h0, h1 = 2 * c, 2 * c + 1
        wq2 = np.ascontiguousarray(
            np.concatenate([W_Q[h0] * scale, W_Q[h1] * scale], axis=1)
        ).astype(np.float32)
        wk2 = np.ascontiguousarray(np.concatenate([W_K[h0], W_K[h1]], axis=1))
        wv2 = np.ascontiguousarray(np.concatenate([W_V[h0], W_V[h1]], axis=1))
        wo2 = np.ascontiguousarray(W_O[P * c : P * (c + 1), :])
        in_maps.append(
            {
                "qT": qTh,
                "kT": kTh,
                "vT": vTh,
                "wq": wq2,
                "wk": wk2,
                "wv": wv2,
                "wo": wo2,
                "maskf": maskf,
                "maskb": maskb,
            }
        )

    trace = bool(int(os.environ.get("ATTN_TRACE", "0")))
    res = run_bass_kernel_spmd(
        nc, in_maps, core_ids=list(range(NCORES)), trace=trace
    )
    LAST_EXEC_NS = res.exec_time_ns

    out = np.zeros((S, D), dtype=np.float32)
    for c in range(NCORES):
        out += res.results[c]["y0"]
        out += res.results[c]["y1"]
    return out
